# revision 35
# baseline (speedup 1.0000x reference)
"""Multi-head causal attention (SEQ=4096, D=1024, H=16, DK=64) on 8 TRN2
NeuronCores, tensor-parallel over heads (2 heads/core). Self-contained.

v1 pipeline (per core, 2 heads):
  One fused nb-loop (512-seq blocks) that interleaves everything so the PE
  never idles and the HAM clock gate stays at 8/8:
    per nb: Qproj(nb), Kproj(nb) | st3(h1, nb-1), scale(nb-1) | Vproj(nb) |
            stats(h0, nb), stats(h1, nb) | st3(h0, nb) | W_O(nb-1)
  - Projections: W.T @ X^T streamed from DRAM (f32r; V in bf16).
  - Stats pass: S = Qh^T.T @ Kh^T (f32r, pad-128 contraction) -> bf16 PSUM
    pairs ([P,1024] = 1 bank) -> one DVE reduce_max per pair (bf16 2x mode).
    Row max -m DMA'd into the rider row (row 64) of qhT.
  - S^T pass: [Kh;1].T @ [Qh;-m] (f32r 1-pass, the -m rider makes exp input
    bounded) -> bf16 PSUM pairs -> one ACT exp per pair -> P^T bf16 ->
    AV accumulate in f32 PSUM with ones-augmented bf16 Vh giving l as row 64.
  - 1/l via reciprocal_approx_fast (single custom-DVE op), R = rank-2
    broadcast over head halves (PE), ct scaled (DVE), Y = ct.T @ W_O_rows,
    bf16 partials DMA'd out; host sums 8 partials in fp32.
"""

import os
import sys

sys.path.insert(0, "/opt/trn_rl_repo")

import numpy as np
import ml_dtypes

import concourse.bass as bass
import concourse.mybir as mybir
import concourse.tile as tile
from concourse.bass_utils import run_bass_kernel_spmd
from concourse.masks import make_identity

P = 128
S = 4096
D = 1024
DK = 64
NH = 2  # heads per core
NCORES = 8
NEG = -1.0e9
F32 = mybir.dt.float32
F32R = mybir.dt.float32r
BF16 = mybir.dt.bfloat16
EXP = mybir.ActivationFunctionType.Exp

_ctr = [0]


def _split_waits(nc, max_waits=1):
    """walrus rejects >1 sem-wait per instruction; move extras onto
    preceding same-engine NOPs (engine streams are program-ordered)."""
    for f in nc.m.functions:
        for bb in f.blocks:
            insts = bb.instructions
            new = []
            changed = False
            for inst in insts:
                si = inst.sync_info
                if si is not None and si.on_wait and len(si.on_wait) > max_waits:
                    waits = list(si.on_wait)
                    extra, keep = waits[:-max_waits], waits[-max_waits:]
                    for i in range(0, len(extra), max_waits):
                        _ctr[0] += 1
                        new.append(
                            mybir.InstNoOp(
                                name=f"waitsplit-{_ctr[0]}",
                                engine=inst.engine,
                                ins=[],
                                outs=[],
                                sync_info=mybir.SyncInfo(
                                    on_wait=extra[i : i + max_waits], on_update=[]
                                ),
                            )
                        )
                    inst.sync_info = mybir.SyncInfo(
                        on_wait=keep, on_update=list(si.on_update)
                    )
                    changed = True
                new.append(inst)
            if changed:
                bb.instructions = new


def build(nc: bass.Bass, causal: bool = True):
    repeat = int(os.environ.get("ATTN_REPEAT", "1"))
    bf_stat = bool(int(os.environ.get("ATTN_BF16_STAT", "1")))
    bf_st = bool(int(os.environ.get("ATTN_BF16_ST", "1")))
    bf_v = bool(int(os.environ.get("ATTN_BF16_V", "1")))
    bf_y = bool(int(os.environ.get("ATTN_BF16_Y", "1")))
    recip_fast = bool(int(os.environ.get("ATTN_RECIP_FAST", "1")))
    pad128 = bool(int(os.environ.get("ATTN_PAD128", "1")))
    # tensor_mask_reduce fails this walrus build's codegen ("ISA wrong
    # length") - keep the mask-matmul path by default.
    maskred = bool(int(os.environ.get("ATTN_MASKRED", "0")))
    affsel = bool(int(os.environ.get("ATTN_AFFSEL", "1")))
    NB = S // 512  # 8   512-wide blocks
    QB = S // P  # 32  128-wide q blocks
    DC = D // P  # 8   128-deep contraction chunks
    FV = BF16 if bf_v else F32R  # vT/wv DMA dtype
    FP = BF16 if (bf_st or bf_v) else F32R  # pt + vh compute dtype
    FY = BF16 if bf_y else F32

    qT = nc.dram_tensor("qT", [D, S], F32R, kind="ExternalInput")
    kT = nc.dram_tensor("kT", [D, S], F32R, kind="ExternalInput")
    vT = nc.dram_tensor("vT", [D, S], FV, kind="ExternalInput")
    wq = nc.dram_tensor("wq", [D, NH * DK], F32R, kind="ExternalInput")
    wk = nc.dram_tensor("wk", [D, NH * DK], F32R, kind="ExternalInput")
    wv = nc.dram_tensor("wv", [D, NH * DK], FV, kind="ExternalInput")
    wo = nc.dram_tensor("wo", [NH * DK, D], F32R, kind="ExternalInput")
    maskend = nc.dram_tensor("maskend", [P, 5], F32, kind="ExternalInput")
    maskf = nc.dram_tensor("maskf", [P, P], BF16, kind="ExternalInput")
    maskb = nc.dram_tensor("maskb", [P, P], BF16, kind="ExternalInput")
    onesr = nc.dram_tensor("onesr", [1, S], F32R, kind="ExternalInput")
    sel2d = nc.dram_tensor("sel2d", [33, P], F32R, kind="ExternalInput")
    zeros64 = nc.dram_tensor("zeros64", [DK, S], F32R, kind="ExternalInput")
    y0 = nc.dram_tensor("y0", [S, D], FY, kind="ExternalOutput")

    with tile.TileContext(nc) as tc:
        import contextlib

        ctx = contextlib.ExitStack()
        with ctx:
            const = ctx.enter_context(tc.tile_pool(name="const", bufs=1))
            big = ctx.enter_context(tc.tile_pool(name="big", bufs=1))
            stream = ctx.enter_context(
                tc.tile_pool(name="stream", bufs=int(os.environ.get("ATTN_BSTREAM", "14")))
            )
            ptp = ctx.enter_context(
                tc.tile_pool(name="ptp", bufs=int(os.environ.get("ATTN_BPT", "3")))
            )
            ypool = ctx.enter_context(
                tc.tile_pool(name="ypool", bufs=int(os.environ.get("ATTN_BY", "2")))
            )
            smalls = ctx.enter_context(tc.tile_pool(name="smalls", bufs=2))
            bproj = int(os.environ.get("ATTN_BPROJ", "2"))
            bstat = int(os.environ.get("ATTN_BSTAT", "2"))
            bst = int(os.environ.get("ATTN_BST", "3"))
            bmisc = int(os.environ.get("ATTN_BMISC", "1"))
            ps_proj = ctx.enter_context(
                tc.tile_pool(name="ps_proj", bufs=bproj, space="PSUM")
            )
            ps_stat = ctx.enter_context(
                tc.tile_pool(name="ps_stat", bufs=bstat, space="PSUM")
            )
            ps_st = ctx.enter_context(tc.tile_pool(name="ps_st", bufs=bst, space="PSUM"))
            ps_misc = ctx.enter_context(
                tc.tile_pool(name="ps_misc", bufs=bmisc, space="PSUM")
            )

            # ---- constants ----
            ident = const.tile([P, P], F32)
            make_identity(nc, ident[:])
            ident_b = const.tile([P, P], BF16)
            nc.vector.tensor_copy(ident_b[:], ident[:])
            ident_r = const.tile([P, P], F32R)
            nc.vector.tensor_copy(ident_r[:], ident[:])
            ident_v = ident_b if FP == BF16 else ident_r

            # wq/wk first on the sync (input-stream) queue so the first proj
            # matmuls start ASAP; everything else on the ACT-side queue.
            wq_sb = const.tile([P, DC, P], F32R, tag="wq")
            wk_sb = const.tile([P, DC, P], F32R, tag="wk")
            wv_sb = const.tile([P, DC, P], FV, tag="wv")
            nc.sync.dma_start(wq_sb[:], wq.rearrange("(o p) m -> p o m", p=P))
            nc.sync.dma_start(wk_sb[:], wk.rearrange("(o p) m -> p o m", p=P))
            nc.scalar.dma_start(wv_sb[:], wv.rearrange("(o p) m -> p o m", p=P))
            wo_sb = const.tile([P, D], F32R, tag="wo")
            nc.scalar.dma_start(wo_sb[:], wo[:])

            me_sb = const.tile([P, 5], F32, tag="me")
            nc.scalar.dma_start(me_sb[:], maskend[:])
            # full-size side-output of tensor_mask_reduce (never read)
            mskscr = const.tile([P, 512], F32, tag="mskscr")
            mf_sb = const.tile([P, P], BF16, tag="mf")
            mb_sb = const.tile([P, P], BF16, tag="mb")
            if not affsel:
                nc.scalar.dma_start(mf_sb[:], maskf[:])
            if not maskred:
                nc.scalar.dma_start(mb_sb[:], maskb[:])

            # ---- persistent activations ----
            qhT = [big.tile([P, S], F32R, tag=f"qhT{h}", name=f"qhT{h}") for h in range(NH)]
            khT = [big.tile([P, S], F32R, tag=f"khT{h}", name=f"khT{h}") for h in range(NH)]
            vh = [big.tile([P, QB, DK + 1], FP, tag=f"vh{h}", name=f"vh{h}") for h in range(NH)]
            ct = big.tile([P, S], F32R, tag="ct")
            mcol = [big.tile([P, QB], F32R, tag=f"mcol{h}", name=f"mcol{h}") for h in range(NH)]
            # 1/l rows (rows 0 / 32): consumed by the rank-2 R matmul.
            lrT = big.tile([33, S], F32R, tag="lrT")

            ones_qb = const.tile([P, QB], F32, tag="ones_qb")
            nc.any.memset(ones_qb[:], 1.0)
            # head-half selector for the rank-2 R matmul: row 0 ->
            # output partitions 0..63, row 32 -> 64..127; rows 1..31 zero.
            sel2 = const.tile([P, P], F32R, tag="sel2")
            nc.scalar.dma_start(sel2[0:33, :], sel2d[:])
            # rows 1..31 of lrT ride into the R matmul as moving junk x zero
            # stationary - zero them so no NaN poisons 0*x.
            nc.scalar.dma_start(lrT[0:33, :], zeros64[0:33, :])
            for h in range(NH):
                if pad128:
                    # zero pad rows once, on the ACT-side DMA queue.
                    # (memset output trips the f32r rounding check, so DMA
                    # host zeros instead.)
                    nc.scalar.dma_start(qhT[h][DK:P, :], zeros64[:])
                    nc.scalar.dma_start(khT[h][DK:P, :], zeros64[:])
                nc.sync.dma_start(khT[h][DK : DK + 1, :], onesr[:])  # ones row
                if FP == BF16:
                    nc.gpsimd.memset(vh[h][:, :, DK], 1.0)  # ones col
                else:
                    nc.vector.tensor_copy(vh[h][:, :, DK], ones_qb[:])

            SPAN = P if pad128 else DK

            def proj_emit(t_idx, nb):
                xdram, w_sb = [(qT, wq_sb), (kT, wk_sb)][t_idx]
                ps = ps_proj.tile([P, 512], F32, tag="proj", name="ps")
                for dc in range(DC):
                    xt = stream.tile([P, 512], F32R, tag="xin", name="xt")
                    nc.sync.dma_start(
                        xt[:],
                        xdram[dc * P : (dc + 1) * P, nb * 512 : (nb + 1) * 512],
                    )
                    nc.tensor.matmul(
                        ps[:],
                        w_sb[:, dc, :],
                        xt[:],
                        start=(dc == 0),
                        stop=(dc == DC - 1),
                    )
                hi_t = qhT if t_idx == 0 else khT
                for h in range(NH):
                    sl = slice(nb * 512, (nb + 1) * 512)
                    nc.scalar.copy(hi_t[h][0:DK, sl], ps[h * DK : (h + 1) * DK, :])

            def vproj_emit(nb):
                ps = ps_proj.tile([P, 512], F32, tag="proj", name="ps")
                for dc in range(DC):
                    xt = stream.tile([P, 512], FV, tag="xin", name="xtv")
                    nc.sync.dma_start(
                        xt[:],
                        vT[dc * P : (dc + 1) * P, nb * 512 : (nb + 1) * 512],
                    )
                    nc.tensor.matmul(
                        ps[:],
                        wv_sb[:, dc, :],
                        xt[:],
                        start=(dc == 0),
                        stop=(dc == DC - 1),
                    )
                vtmp = stream.tile([P, 512], FP, tag="xin", name="vtmp")
                nc.vector.tensor_copy(vtmp[:], ps[:])
                for h in range(NH):
                    pst = ps_misc.tile([P, 512], FP, tag="misc", name="pst")
                    for j in range(4):
                        nc.tensor.transpose(
                            pst[0:P, j * DK : (j + 1) * DK],
                            vtmp[h * DK : (h + 1) * DK, j * P : (j + 1) * P],
                            ident_v[h * DK : (h + 1) * DK, h * DK : (h + 1) * DK],
                        )
                    nc.vector.tensor_copy(
                        vh[h][:, nb * 4 : nb * 4 + 4, 0:DK],
                        pst[:, 0 : 4 * DK].rearrange("p (j d) -> p j d", j=4),
                    )

            # ---- stats: one S block + masked row-max per unit (weavable).
            # The causal diagonal is handled by the mask_end column of the
            # chained tensor_mask_reduce (no mask matmul), and the running
            # max accumulates across blocks via accum_in/accum_out.
            FMIN = float(np.finfo(np.float32).min)

            def stats_units(h, qbs):
                """Yield closures, each emitting ~1 PE matmul of stats work."""
                for qb in qbs:
                    kmax = qb // 4 + 1 if causal else NB
                    state = {}

                    def mk(kc, h=h, qb=qb, kmax=kmax, state=state):
                        def emit():
                            if kc == 0:
                                state["macc"] = smalls.tile(
                                    [P, 1], F32, tag="macc", name="macc"
                                )
                                if not maskred:
                                    state["mpart"] = smalls.tile(
                                        [P, NB], F32, tag="mpart", name="mpart"
                                    )
                            macc = state["macc"]
                            ps = ps_stat.tile([P, 512], F32, tag="stat", name="ps_stat")
                            diag = causal and (kc == qb // 4)
                            nv = (qb % 4) * P + P if diag else 512
                            last = kc == kmax - 1
                            nc.tensor.matmul(
                                ps[:, 0:nv],
                                qhT[h][0:SPAN, qb * P : (qb + 1) * P],
                                khT[h][0:SPAN, kc * 512 : kc * 512 + nv],
                                start=True,
                                stop=not (diag and not maskred),
                            )
                            if maskred:
                                me = (
                                    me_sb[:, (qb % 4) : (qb % 4) + 1]
                                    if diag
                                    else me_sb[:, 4:5]
                                )
                                nc.vector.tensor_mask_reduce(
                                    out=mskscr[:, 0:nv],
                                    in_=ps[:, 0:nv],
                                    mask_start=0.0,
                                    mask_end=me,
                                    scale=1.0,
                                    accum_in=(FMIN if kc == 0 else macc[:]),
                                    op=mybir.AluOpType.max,
                                    negate_accum=last,
                                    accum_out=macc[:],
                                )
                            else:
                                if diag:
                                    nc.tensor.matmul(
                                        ps[:, nv - P : nv],
                                        ident_b[:],
                                        mb_sb[:],
                                        start=False,
                                        stop=True,
                                    )
                                mpart = state["mpart"]
                                nc.vector.reduce_max(
                                    mpart[:, kc : kc + 1],
                                    ps[:, 0:nv],
                                    axis=mybir.AxisListType.X,
                                )
                                if last:
                                    nc.vector.tensor_reduce(
                                        macc[:],
                                        mpart[:, 0:kmax],
                                        axis=mybir.AxisListType.X,
                                        op=mybir.AluOpType.max,
                                        negate=True,
                                    )
                            if last:
                                # cast -m to f32r on the idle GpSimd engine,
                                # then DMA into the rider row.
                                nc.gpsimd.tensor_copy(
                                    mcol[h][:, qb : qb + 1], macc[:]
                                )
                                nc.sync.dma_start(
                                    qhT[h][DK : DK + 1, qb * P : (qb + 1) * P],
                                    mcol[h][:, qb : qb + 1],
                                )

                        return emit

                    for kc in range(kmax):
                        yield mk(kc)

            # ---- S^T + exp + AV for one (head, 512-q block) ----
            # fillers: independent ~1-matmul closures woven between the
            # S^T matmul of block kc+1 and the AV of block kc so the PE
            # stays busy during the (longer) ACT exp of block kc.
            def st3_emit(h, nb, fillers=()):
                nkc = 4 * (nb + 1) if causal else QB
                fillers = list(fillers)
                nf = len(fillers)
                fi = 0
                po = ps_misc.tile([P, 512], F32, tag="misc", name="po")
                pss = {}

                def s_mm(kc):
                    ps = ps_st.tile([P, 512], F32, tag="st", name="ps_st")
                    diag = causal and (kc >= 4 * nb)
                    o = kc - 4 * nb if diag else 0
                    qoff = o * P
                    nv = 512 - qoff
                    kslice = slice(kc * P, (kc + 1) * P)
                    qslice = slice(nb * 512 + qoff, (nb + 1) * 512)
                    span = P if pad128 else DK + 1
                    nc.tensor.matmul(
                        ps[:, 0:nv],
                        khT[h][0:span, kslice],
                        qhT[h][0:span, qslice],
                        start=True,
                        stop=not (diag and not affsel),
                    )
                    if diag and not affsel:
                        nc.tensor.matmul(
                            ps[:, 0:P],
                            ident_b[:],
                            mf_sb[:],
                            start=False,
                            stop=True,
                        )
                    pss[kc] = (ps, qoff, nv, diag)

                s_mm(0)
                for kc in range(nkc):
                    if kc + 1 < nkc:
                        s_mm(kc + 1)
                    ps, qoff, nv, diag = pss.pop(kc)
                    pt = ptp.tile([P, 512], FP, tag="pt", name="pt")
                    nc.scalar.activation(pt[:, 0:nv], ps[:, 0:nv], EXP)
                    if diag and affsel:
                        # zero the future (kc > q) triangle of the first 128
                        # q-cols on the idle GpSimd engine (replaces the
                        # -1e9 mask matmul; exp may have produced Inf there).
                        nc.gpsimd.affine_select(
                            pt[:, 0:P],
                            pt[:, 0:P],
                            pattern=[[1, P]],
                            compare_op=mybir.AluOpType.is_ge,
                            fill=0.0,
                            base=0,
                            channel_multiplier=-1,
                        )
                    # evenly spread the fillers across the kc slots
                    want = (kc + 1) * nf // nkc
                    while fi < want:
                        fillers[fi]()
                        fi += 1
                    nc.tensor.matmul(
                        po[0 : DK + 1, qoff:512],
                        vh[h][:, kc, :],
                        pt[:, 0:nv],
                        start=(kc == 0),
                        stop=(kc == nkc - 1),
                    )
                nc.scalar.copy(
                    ct[h * DK : (h + 1) * DK, nb * 512 : (nb + 1) * 512],
                    po[0:DK, :],
                )
                lrow = 32 * h
                if recip_fast:
                    # lrT holds plain l; scale_emit broadcasts it and takes
                    # the reciprocal there in a well-shaped [128,512] op.
                    nc.scalar.copy(
                        lrT[lrow : lrow + 1, nb * 512 : (nb + 1) * 512],
                        po[DK : DK + 1, :],
                    )
                else:
                    with nc.allow_low_precision(reason="1/l scale"):
                        nc.vector.reciprocal(
                            lrT[lrow : lrow + 1, nb * 512 : (nb + 1) * 512],
                            po[DK : DK + 1, :],
                        )

            def scale_emit(nbv):
                # ct[:, nbv*512:(nbv+1)*512] *= 1/l broadcast over head
                # halves: rank-2 matmul sel2 x lrT -> [128, 512] in PSUM
                # (l or 1/l per recip_fast) -> recip (if needed) -> DVE mul.
                sl = slice(nbv * 512, (nbv + 1) * 512)
                Rps = ps_misc.tile([P, 512], F32, tag="misc", name="Rps")
                nc.tensor.matmul(
                    Rps[:], sel2[0:33, :], lrT[0:33, sl], start=True, stop=True
                )
                if recip_fast:
                    # one [128,512] reciprocal on the broadcast (vs two
                    # [1,512] rows), then the scale mul.
                    rin = smalls.tile([P, 512], F32, tag="rinv", name="rinv")
                    nc.vector.reciprocal(rin[:], Rps[:])
                    nc.vector.tensor_mul(out=ct[:, sl], in0=ct[:, sl], in1=rin[:])
                else:
                    nc.vector.tensor_mul(out=ct[:, sl], in0=ct[:, sl], in1=Rps[:])

            def wo_units(nbv):
                """Yield 8 closures (one matmul each) for the 4 q-chunks of
                512-q block nbv; ct[:, nbv] must already be scaled."""
                for j in range(4):
                    qc = nbv * 4 + j
                    state = {}

                    def mk(eb, qc=qc, state=state):
                        def emit():
                            if eb == 0:
                                state["ysb"] = ypool.tile(
                                    [P, D], FY, tag="ysb", name="ysb"
                                )
                            ysb = state["ysb"]
                            psy = ps_proj.tile([P, 512], F32, tag="proj", name="psy")
                            nc.tensor.matmul(
                                psy[:],
                                ct[:, qc * P : (qc + 1) * P],
                                wo_sb[:, eb * 512 : (eb + 1) * 512],
                                start=True,
                                stop=True,
                            )
                            # split PSUM->SBUF copies across DVE and ACT
                            if eb == 0:
                                nc.vector.tensor_copy(ysb[:, 0:512], psy[:])
                            else:
                                nc.scalar.copy(ysb[:, 512:1024], psy[:])
                                nc.sync.dma_start(
                                    y0[qc * P : (qc + 1) * P, :], ysb[:]
                                )

                        return emit

                    yield mk(0)
                    yield mk(1)

            for _rep in range(repeat):
                for nb in range(NB):
                    proj_emit(0, nb)
                    proj_emit(1, nb)
                    vproj_emit(nb)
                    if nb == 0:
                        for u in stats_units(0, range(0, 4)):
                            u()
                        st3_emit(0, 0, fillers=stats_units(1, range(0, 4)))
                    else:
                        fl = []
                        if nb >= 2:
                            fl += list(wo_units(nb - 2))
                        fl += list(stats_units(0, range(4 * nb, 4 * nb + 4)))
                        st3_emit(1, nb - 1, fillers=fl)
                        scale_emit(nb - 1)
                        st3_emit(
                            0, nb, fillers=stats_units(1, range(4 * nb, 4 * nb + 4))
                        )
                st3_emit(1, NB - 1, fillers=wo_units(NB - 2))
                scale_emit(NB - 1)
                for u in wo_units(NB - 1):
                    u()

    _split_waits(nc)
    return nc


_cache = {}


def _get_nc(causal: bool):
    if causal not in _cache:
        nc = bass.Bass(trn_type="TRN2")
        build(nc, causal=causal)
        _cache[causal] = nc
    return _cache[causal]


def _host_maskend():
    # mask_end columns for the stats tensor_mask_reduce: col j<4 = causal
    # end (exclusive) of the diag block's valid range when qb%4 == j;
    # col 4 = 512 (full block).
    p = np.arange(P)
    me = np.zeros((P, 5), dtype=np.float32)
    for j in range(4):
        me[:, j] = j * P + p + 1
    me[:, 4] = 512.0
    return me


def make_in_maps(np_inputs):
    bf_v = bool(int(os.environ.get("ATTN_BF16_V", "1")))
    Q = np.asarray(np_inputs["Q"], dtype=np.float32)
    K = np.asarray(np_inputs["K"], dtype=np.float32)
    V = np.asarray(np_inputs["V"], dtype=np.float32)
    W_Q = np.asarray(np_inputs["W_Q"], dtype=np.float32)
    W_K = np.asarray(np_inputs["W_K"], dtype=np.float32)
    W_V = np.asarray(np_inputs["W_V"], dtype=np.float32)
    W_O = np.asarray(np_inputs["W_O"], dtype=np.float32)

    qTh = np.ascontiguousarray(Q.T)
    kTh = np.ascontiguousarray(K.T)
    vTh = np.ascontiguousarray(V.T)
    if bf_v:
        vTh = vTh.astype(ml_dtypes.bfloat16)
    maskend = _host_maskend()
    p_ = np.arange(P)[:, None]
    j_ = np.arange(P)[None, :]
    maskf = np.where(p_ > j_, NEG, 0.0).astype(ml_dtypes.bfloat16)
    maskb = np.where(j_ > p_, NEG, 0.0).astype(ml_dtypes.bfloat16)
    ones_row = np.ones((1, S), dtype=np.float32)
    sel2 = np.zeros((33, P), dtype=np.float32)
    sel2[0, 0:DK] = 1.0
    sel2[32, DK:P] = 1.0

    scale = np.float32(1.0 / np.sqrt(DK))
    in_maps = []
    for c in range(NCORES):
        h0, h1 = 2 * c, 2 * c + 1
        wq2 = np.ascontiguousarray(
            np.concatenate([W_Q[h0] * scale, W_Q[h1] * scale], axis=1)
        ).astype(np.float32)
        wk2 = np.ascontiguousarray(np.concatenate([W_K[h0], W_K[h1]], axis=1))
        wv2 = np.ascontiguousarray(np.concatenate([W_V[h0], W_V[h1]], axis=1))
        if bf_v:
            wv2 = wv2.astype(ml_dtypes.bfloat16)
        wo2 = np.ascontiguousarray(W_O[P * c : P * (c + 1), :])
        in_maps.append(
            {
                "qT": qTh,
                "kT": kTh,
                "vT": vTh,
                "wq": wq2,
                "wk": wk2,
                "wv": wv2,
                "wo": wo2,
                "maskend": maskend,
                "maskf": maskf,
                "maskb": maskb,
                "onesr": ones_row,
                "sel2d": sel2,
                "zeros64": np.zeros((DK, S), dtype=np.float32),
            }
        )
    return in_maps


LAST_EXEC_NS = None


def kernel(Q, K, V, W_Q, W_K, W_V, W_O, mask):
    global LAST_EXEC_NS
    causal = bool(np.asarray(mask).item())
    nc = _get_nc(causal)
    in_maps = make_in_maps(
        dict(Q=Q, K=K, V=V, W_Q=W_Q, W_K=W_K, W_V=W_V, W_O=W_O)
    )

    trace = bool(int(os.environ.get("ATTN_TRACE", "0")))
    res = run_bass_kernel_spmd(
        nc, in_maps, core_ids=list(range(NCORES)), trace=trace
    )
    LAST_EXEC_NS = res.exec_time_ns

    out = np.zeros((S, D), dtype=np.float32)
    for c in range(NCORES):
        out += np.asarray(res.results[c]["y0"], dtype=np.float32)
    return out


# revision 36
# speedup vs baseline: 1.0216x; 1.0216x over previous
"""Multi-head causal attention (SEQ=4096, D=1024, H=16, DK=64) on 8 TRN2
NeuronCores, tensor-parallel over heads (2 heads/core). Self-contained.

v1 pipeline (per core, 2 heads):
  One fused nb-loop (512-seq blocks) that interleaves everything so the PE
  never idles and the HAM clock gate stays at 8/8:
    per nb: Qproj(nb), Kproj(nb) | st3(h1, nb-1), scale(nb-1) | Vproj(nb) |
            stats(h0, nb), stats(h1, nb) | st3(h0, nb) | W_O(nb-1)
  - Projections: W.T @ X^T streamed from DRAM (f32r; V in bf16).
  - Stats pass: S = Qh^T.T @ Kh^T (f32r, pad-128 contraction) -> bf16 PSUM
    pairs ([P,1024] = 1 bank) -> one DVE reduce_max per pair (bf16 2x mode).
    Row max -m DMA'd into the rider row (row 64) of qhT.
  - S^T pass: [Kh;1].T @ [Qh;-m] (f32r 1-pass, the -m rider makes exp input
    bounded) -> bf16 PSUM pairs -> one ACT exp per pair -> P^T bf16 ->
    AV accumulate in f32 PSUM with ones-augmented bf16 Vh giving l as row 64.
  - 1/l via reciprocal_approx_fast (single custom-DVE op), R = rank-2
    broadcast over head halves (PE), ct scaled (DVE), Y = ct.T @ W_O_rows,
    bf16 partials DMA'd out; host sums 8 partials in fp32.
"""

import os
import sys

sys.path.insert(0, "/opt/trn_rl_repo")

import numpy as np
import ml_dtypes

import concourse.bass as bass
import concourse.mybir as mybir
import concourse.tile as tile
from concourse.bass_utils import run_bass_kernel_spmd
from concourse.masks import make_identity

P = 128
S = 4096
D = 1024
DK = 64
NH = 2  # heads per core
NCORES = 8
NEG = -1.0e9
F32 = mybir.dt.float32
F32R = mybir.dt.float32r
BF16 = mybir.dt.bfloat16
EXP = mybir.ActivationFunctionType.Exp

_ctr = [0]


def _split_waits(nc, max_waits=1):
    """walrus rejects >1 sem-wait per instruction; move extras onto
    preceding same-engine NOPs (engine streams are program-ordered)."""
    for f in nc.m.functions:
        for bb in f.blocks:
            insts = bb.instructions
            new = []
            changed = False
            for inst in insts:
                si = inst.sync_info
                if si is not None and si.on_wait and len(si.on_wait) > max_waits:
                    waits = list(si.on_wait)
                    extra, keep = waits[:-max_waits], waits[-max_waits:]
                    for i in range(0, len(extra), max_waits):
                        _ctr[0] += 1
                        new.append(
                            mybir.InstNoOp(
                                name=f"waitsplit-{_ctr[0]}",
                                engine=inst.engine,
                                ins=[],
                                outs=[],
                                sync_info=mybir.SyncInfo(
                                    on_wait=extra[i : i + max_waits], on_update=[]
                                ),
                            )
                        )
                    inst.sync_info = mybir.SyncInfo(
                        on_wait=keep, on_update=list(si.on_update)
                    )
                    changed = True
                new.append(inst)
            if changed:
                bb.instructions = new


def build(nc: bass.Bass, causal: bool = True):
    repeat = int(os.environ.get("ATTN_REPEAT", "1"))
    bf_stat = bool(int(os.environ.get("ATTN_BF16_STAT", "1")))
    bf_st = bool(int(os.environ.get("ATTN_BF16_ST", "1")))
    bf_v = bool(int(os.environ.get("ATTN_BF16_V", "1")))
    bf_y = bool(int(os.environ.get("ATTN_BF16_Y", "1")))
    recip_fast = bool(int(os.environ.get("ATTN_RECIP_FAST", "1")))
    pad128 = bool(int(os.environ.get("ATTN_PAD128", "1")))
    # tensor_mask_reduce fails this walrus build's codegen ("ISA wrong
    # length") - keep the mask-matmul path by default.
    maskred = bool(int(os.environ.get("ATTN_MASKRED", "0")))
    # the GpSimd zeroing sits on the exp->AV critical chain and measured
    # slower than the -1e9 mask matmul; keep the matmul path by default.
    affsel = bool(int(os.environ.get("ATTN_AFFSEL", "0")))
    NB = S // 512  # 8   512-wide blocks
    QB = S // P  # 32  128-wide q blocks
    DC = D // P  # 8   128-deep contraction chunks
    FV = BF16 if bf_v else F32R  # vT/wv DMA dtype
    FP = BF16 if (bf_st or bf_v) else F32R  # pt + vh compute dtype
    FY = BF16 if bf_y else F32

    qT = nc.dram_tensor("qT", [D, S], F32R, kind="ExternalInput")
    kT = nc.dram_tensor("kT", [D, S], F32R, kind="ExternalInput")
    vT = nc.dram_tensor("vT", [D, S], FV, kind="ExternalInput")
    wq = nc.dram_tensor("wq", [D, NH * DK], F32R, kind="ExternalInput")
    wk = nc.dram_tensor("wk", [D, NH * DK], F32R, kind="ExternalInput")
    wv = nc.dram_tensor("wv", [D, NH * DK], FV, kind="ExternalInput")
    wo = nc.dram_tensor("wo", [NH * DK, D], F32R, kind="ExternalInput")
    maskend = nc.dram_tensor("maskend", [P, 5], F32, kind="ExternalInput")
    maskf = nc.dram_tensor("maskf", [P, P], BF16, kind="ExternalInput")
    maskb = nc.dram_tensor("maskb", [P, P], BF16, kind="ExternalInput")
    onesr = nc.dram_tensor("onesr", [1, S], F32R, kind="ExternalInput")
    sel2d = nc.dram_tensor("sel2d", [33, P], F32R, kind="ExternalInput")
    zeros64 = nc.dram_tensor("zeros64", [DK, S], F32R, kind="ExternalInput")
    y0 = nc.dram_tensor("y0", [S, D], FY, kind="ExternalOutput")

    with tile.TileContext(nc) as tc:
        import contextlib

        ctx = contextlib.ExitStack()
        with ctx:
            const = ctx.enter_context(tc.tile_pool(name="const", bufs=1))
            big = ctx.enter_context(tc.tile_pool(name="big", bufs=1))
            stream = ctx.enter_context(
                tc.tile_pool(name="stream", bufs=int(os.environ.get("ATTN_BSTREAM", "14")))
            )
            ptp = ctx.enter_context(
                tc.tile_pool(name="ptp", bufs=int(os.environ.get("ATTN_BPT", "3")))
            )
            ypool = ctx.enter_context(
                tc.tile_pool(name="ypool", bufs=int(os.environ.get("ATTN_BY", "2")))
            )
            smalls = ctx.enter_context(tc.tile_pool(name="smalls", bufs=2))
            bproj = int(os.environ.get("ATTN_BPROJ", "2"))
            bstat = int(os.environ.get("ATTN_BSTAT", "2"))
            bst = int(os.environ.get("ATTN_BST", "3"))
            bmisc = int(os.environ.get("ATTN_BMISC", "1"))
            ps_proj = ctx.enter_context(
                tc.tile_pool(name="ps_proj", bufs=bproj, space="PSUM")
            )
            ps_stat = ctx.enter_context(
                tc.tile_pool(name="ps_stat", bufs=bstat, space="PSUM")
            )
            ps_st = ctx.enter_context(tc.tile_pool(name="ps_st", bufs=bst, space="PSUM"))
            ps_misc = ctx.enter_context(
                tc.tile_pool(name="ps_misc", bufs=bmisc, space="PSUM")
            )

            # ---- constants ----
            ident = const.tile([P, P], F32)
            make_identity(nc, ident[:])
            ident_b = const.tile([P, P], BF16)
            nc.vector.tensor_copy(ident_b[:], ident[:])
            ident_r = const.tile([P, P], F32R)
            nc.vector.tensor_copy(ident_r[:], ident[:])
            ident_v = ident_b if FP == BF16 else ident_r

            # wq/wk first on the sync (input-stream) queue so the first proj
            # matmuls start ASAP; everything else on the ACT-side queue.
            wq_sb = const.tile([P, DC, P], F32R, tag="wq")
            wk_sb = const.tile([P, DC, P], F32R, tag="wk")
            wv_sb = const.tile([P, DC, P], FV, tag="wv")
            nc.sync.dma_start(wq_sb[:], wq.rearrange("(o p) m -> p o m", p=P))
            nc.sync.dma_start(wk_sb[:], wk.rearrange("(o p) m -> p o m", p=P))
            nc.scalar.dma_start(wv_sb[:], wv.rearrange("(o p) m -> p o m", p=P))
            wo_sb = const.tile([P, D], F32R, tag="wo")
            nc.scalar.dma_start(wo_sb[:], wo[:])

            me_sb = const.tile([P, 5], F32, tag="me")
            nc.scalar.dma_start(me_sb[:], maskend[:])
            # full-size side-output of tensor_mask_reduce (never read)
            mskscr = const.tile([P, 512], F32, tag="mskscr")
            mf_sb = const.tile([P, P], BF16, tag="mf")
            mb_sb = const.tile([P, P], BF16, tag="mb")
            if not affsel:
                nc.scalar.dma_start(mf_sb[:], maskf[:])
            if not maskred:
                nc.scalar.dma_start(mb_sb[:], maskb[:])

            # ---- persistent activations ----
            qhT = [big.tile([P, S], F32R, tag=f"qhT{h}", name=f"qhT{h}") for h in range(NH)]
            khT = [big.tile([P, S], F32R, tag=f"khT{h}", name=f"khT{h}") for h in range(NH)]
            vh = [big.tile([P, QB, DK + 1], FP, tag=f"vh{h}", name=f"vh{h}") for h in range(NH)]
            ct = big.tile([P, S], F32R, tag="ct")
            mcol = [big.tile([P, QB], F32R, tag=f"mcol{h}", name=f"mcol{h}") for h in range(NH)]
            # 1/l rows (rows 0 / 32): consumed by the rank-2 R matmul.
            lrT = big.tile([33, S], F32R, tag="lrT")

            ones_qb = const.tile([P, QB], F32, tag="ones_qb")
            nc.any.memset(ones_qb[:], 1.0)
            # head-half selector for the rank-2 R matmul: row 0 ->
            # output partitions 0..63, row 32 -> 64..127; rows 1..31 zero.
            sel2 = const.tile([P, P], F32R, tag="sel2")
            nc.scalar.dma_start(sel2[0:33, :], sel2d[:])
            # rows 1..31 of lrT ride into the R matmul as moving junk x zero
            # stationary - zero them so no NaN poisons 0*x.
            nc.scalar.dma_start(lrT[0:33, :], zeros64[0:33, :])
            for h in range(NH):
                if pad128:
                    # zero pad rows once, on the ACT-side DMA queue.
                    # (memset output trips the f32r rounding check, so DMA
                    # host zeros instead.)
                    nc.scalar.dma_start(qhT[h][DK:P, :], zeros64[:])
                    nc.scalar.dma_start(khT[h][DK:P, :], zeros64[:])
                nc.sync.dma_start(khT[h][DK : DK + 1, :], onesr[:])  # ones row
                if FP == BF16:
                    nc.gpsimd.memset(vh[h][:, :, DK], 1.0)  # ones col
                else:
                    nc.vector.tensor_copy(vh[h][:, :, DK], ones_qb[:])

            SPAN = P if pad128 else DK

            def proj_emit(t_idx, nb):
                xdram, w_sb = [(qT, wq_sb), (kT, wk_sb)][t_idx]
                ps = ps_proj.tile([P, 512], F32, tag="proj", name="ps")
                for dc in range(DC):
                    xt = stream.tile([P, 512], F32R, tag="xin", name="xt")
                    nc.sync.dma_start(
                        xt[:],
                        xdram[dc * P : (dc + 1) * P, nb * 512 : (nb + 1) * 512],
                    )
                    nc.tensor.matmul(
                        ps[:],
                        w_sb[:, dc, :],
                        xt[:],
                        start=(dc == 0),
                        stop=(dc == DC - 1),
                    )
                hi_t = qhT if t_idx == 0 else khT
                for h in range(NH):
                    sl = slice(nb * 512, (nb + 1) * 512)
                    nc.scalar.copy(hi_t[h][0:DK, sl], ps[h * DK : (h + 1) * DK, :])

            def vproj_emit(nb):
                ps = ps_proj.tile([P, 512], F32, tag="proj", name="ps")
                for dc in range(DC):
                    xt = stream.tile([P, 512], FV, tag="xin", name="xtv")
                    nc.sync.dma_start(
                        xt[:],
                        vT[dc * P : (dc + 1) * P, nb * 512 : (nb + 1) * 512],
                    )
                    nc.tensor.matmul(
                        ps[:],
                        wv_sb[:, dc, :],
                        xt[:],
                        start=(dc == 0),
                        stop=(dc == DC - 1),
                    )
                vtmp = stream.tile([P, 512], FP, tag="xin", name="vtmp")
                nc.vector.tensor_copy(vtmp[:], ps[:])
                for h in range(NH):
                    pst = ps_misc.tile([P, 512], FP, tag="misc", name="pst")
                    for j in range(4):
                        nc.tensor.transpose(
                            pst[0:P, j * DK : (j + 1) * DK],
                            vtmp[h * DK : (h + 1) * DK, j * P : (j + 1) * P],
                            ident_v[h * DK : (h + 1) * DK, h * DK : (h + 1) * DK],
                        )
                    nc.vector.tensor_copy(
                        vh[h][:, nb * 4 : nb * 4 + 4, 0:DK],
                        pst[:, 0 : 4 * DK].rearrange("p (j d) -> p j d", j=4),
                    )

            # ---- stats: one S block + masked row-max per unit (weavable).
            # The causal diagonal is handled by the mask_end column of the
            # chained tensor_mask_reduce (no mask matmul), and the running
            # max accumulates across blocks via accum_in/accum_out.
            FMIN = float(np.finfo(np.float32).min)

            def stats_units(h, qbs):
                """Yield closures, each emitting ~1 PE matmul of stats work."""
                for qb in qbs:
                    kmax = qb // 4 + 1 if causal else NB
                    state = {}

                    def mk(kc, h=h, qb=qb, kmax=kmax, state=state):
                        def emit():
                            if kc == 0:
                                state["macc"] = smalls.tile(
                                    [P, 1], F32, tag="macc", name="macc"
                                )
                                if not maskred:
                                    state["mpart"] = smalls.tile(
                                        [P, NB], F32, tag="mpart", name="mpart"
                                    )
                            macc = state["macc"]
                            ps = ps_stat.tile([P, 512], F32, tag="stat", name="ps_stat")
                            diag = causal and (kc == qb // 4)
                            nv = (qb % 4) * P + P if diag else 512
                            last = kc == kmax - 1
                            nc.tensor.matmul(
                                ps[:, 0:nv],
                                qhT[h][0:SPAN, qb * P : (qb + 1) * P],
                                khT[h][0:SPAN, kc * 512 : kc * 512 + nv],
                                start=True,
                                stop=not (diag and not maskred),
                            )
                            if maskred:
                                me = (
                                    me_sb[:, (qb % 4) : (qb % 4) + 1]
                                    if diag
                                    else me_sb[:, 4:5]
                                )
                                nc.vector.tensor_mask_reduce(
                                    out=mskscr[:, 0:nv],
                                    in_=ps[:, 0:nv],
                                    mask_start=0.0,
                                    mask_end=me,
                                    scale=1.0,
                                    accum_in=(FMIN if kc == 0 else macc[:]),
                                    op=mybir.AluOpType.max,
                                    negate_accum=last,
                                    accum_out=macc[:],
                                )
                            else:
                                if diag:
                                    nc.tensor.matmul(
                                        ps[:, nv - P : nv],
                                        ident_b[:],
                                        mb_sb[:],
                                        start=False,
                                        stop=True,
                                    )
                                mpart = state["mpart"]
                                nc.vector.reduce_max(
                                    mpart[:, kc : kc + 1],
                                    ps[:, 0:nv],
                                    axis=mybir.AxisListType.X,
                                )
                                if last:
                                    nc.vector.tensor_reduce(
                                        macc[:],
                                        mpart[:, 0:kmax],
                                        axis=mybir.AxisListType.X,
                                        op=mybir.AluOpType.max,
                                        negate=True,
                                    )
                            if last:
                                # cast -m to f32r on the idle GpSimd engine,
                                # then DMA into the rider row.
                                nc.gpsimd.tensor_copy(
                                    mcol[h][:, qb : qb + 1], macc[:]
                                )
                                nc.sync.dma_start(
                                    qhT[h][DK : DK + 1, qb * P : (qb + 1) * P],
                                    mcol[h][:, qb : qb + 1],
                                )

                        return emit

                    for kc in range(kmax):
                        yield mk(kc)

            # ---- S^T + exp + AV for one (head, 512-q block) ----
            # fillers: independent ~1-matmul closures woven between the
            # S^T matmul of block kc+1 and the AV of block kc so the PE
            # stays busy during the (longer) ACT exp of block kc.
            def st3_emit(h, nb, fillers=()):
                nkc = 4 * (nb + 1) if causal else QB
                fillers = list(fillers)
                nf = len(fillers)
                fi = 0
                po = ps_misc.tile([P, 512], F32, tag="misc", name="po")
                pss = {}

                def s_mm(kc):
                    ps = ps_st.tile([P, 512], F32, tag="st", name="ps_st")
                    diag = causal and (kc >= 4 * nb)
                    o = kc - 4 * nb if diag else 0
                    qoff = o * P
                    nv = 512 - qoff
                    kslice = slice(kc * P, (kc + 1) * P)
                    qslice = slice(nb * 512 + qoff, (nb + 1) * 512)
                    span = P if pad128 else DK + 1
                    nc.tensor.matmul(
                        ps[:, 0:nv],
                        khT[h][0:span, kslice],
                        qhT[h][0:span, qslice],
                        start=True,
                        stop=not (diag and not affsel),
                    )
                    if diag and not affsel:
                        nc.tensor.matmul(
                            ps[:, 0:P],
                            ident_b[:],
                            mf_sb[:],
                            start=False,
                            stop=True,
                        )
                    pss[kc] = (ps, qoff, nv, diag)

                s_mm(0)
                for kc in range(nkc):
                    if kc + 1 < nkc:
                        s_mm(kc + 1)
                    ps, qoff, nv, diag = pss.pop(kc)
                    pt = ptp.tile([P, 512], FP, tag="pt", name="pt")
                    nc.scalar.activation(pt[:, 0:nv], ps[:, 0:nv], EXP)
                    if diag and affsel:
                        # zero the future (kc > q) triangle of the first 128
                        # q-cols on the idle GpSimd engine (replaces the
                        # -1e9 mask matmul; exp may have produced Inf there).
                        nc.gpsimd.affine_select(
                            pt[:, 0:P],
                            pt[:, 0:P],
                            pattern=[[1, P]],
                            compare_op=mybir.AluOpType.is_ge,
                            fill=0.0,
                            base=0,
                            channel_multiplier=-1,
                        )
                    # evenly spread the fillers across the kc slots
                    want = (kc + 1) * nf // nkc
                    while fi < want:
                        fillers[fi]()
                        fi += 1
                    nc.tensor.matmul(
                        po[0 : DK + 1, qoff:512],
                        vh[h][:, kc, :],
                        pt[:, 0:nv],
                        start=(kc == 0),
                        stop=(kc == nkc - 1),
                    )
                nc.scalar.copy(
                    ct[h * DK : (h + 1) * DK, nb * 512 : (nb + 1) * 512],
                    po[0:DK, :],
                )
                lrow = 32 * h
                if recip_fast:
                    # lrT holds plain l; scale_emit broadcasts it and takes
                    # the reciprocal there in a well-shaped [128,512] op.
                    nc.scalar.copy(
                        lrT[lrow : lrow + 1, nb * 512 : (nb + 1) * 512],
                        po[DK : DK + 1, :],
                    )
                else:
                    with nc.allow_low_precision(reason="1/l scale"):
                        nc.vector.reciprocal(
                            lrT[lrow : lrow + 1, nb * 512 : (nb + 1) * 512],
                            po[DK : DK + 1, :],
                        )

            def scale_emit(nbv):
                # ct[:, nbv*512:(nbv+1)*512] *= 1/l broadcast over head
                # halves: rank-2 matmul sel2 x lrT -> [128, 512] in PSUM
                # (l or 1/l per recip_fast) -> recip (if needed) -> DVE mul.
                sl = slice(nbv * 512, (nbv + 1) * 512)
                Rps = ps_misc.tile([P, 512], F32, tag="misc", name="Rps")
                nc.tensor.matmul(
                    Rps[:], sel2[0:33, :], lrT[0:33, sl], start=True, stop=True
                )
                if recip_fast:
                    # one [128,512] reciprocal on the broadcast (vs two
                    # [1,512] rows), then the scale mul.
                    rin = smalls.tile([P, 512], F32, tag="rinv", name="rinv")
                    nc.vector.reciprocal(rin[:], Rps[:])
                    nc.vector.tensor_mul(out=ct[:, sl], in0=ct[:, sl], in1=rin[:])
                else:
                    nc.vector.tensor_mul(out=ct[:, sl], in0=ct[:, sl], in1=Rps[:])

            def wo_units(nbv):
                """Yield 8 closures (one matmul each) for the 4 q-chunks of
                512-q block nbv; ct[:, nbv] must already be scaled."""
                for j in range(4):
                    qc = nbv * 4 + j
                    state = {}

                    def mk(eb, qc=qc, state=state):
                        def emit():
                            if eb == 0:
                                state["ysb"] = ypool.tile(
                                    [P, D], FY, tag="ysb", name="ysb"
                                )
                            ysb = state["ysb"]
                            psy = ps_proj.tile([P, 512], F32, tag="proj", name="psy")
                            nc.tensor.matmul(
                                psy[:],
                                ct[:, qc * P : (qc + 1) * P],
                                wo_sb[:, eb * 512 : (eb + 1) * 512],
                                start=True,
                                stop=True,
                            )
                            # split PSUM->SBUF copies across DVE and ACT
                            if eb == 0:
                                nc.vector.tensor_copy(ysb[:, 0:512], psy[:])
                            else:
                                nc.scalar.copy(ysb[:, 512:1024], psy[:])
                                nc.sync.dma_start(
                                    y0[qc * P : (qc + 1) * P, :], ysb[:]
                                )

                        return emit

                    yield mk(0)
                    yield mk(1)

            for _rep in range(repeat):
                for nb in range(NB):
                    proj_emit(0, nb)
                    proj_emit(1, nb)
                    vproj_emit(nb)
                    if nb == 0:
                        for u in stats_units(0, range(0, 4)):
                            u()
                        st3_emit(0, 0, fillers=stats_units(1, range(0, 4)))
                    else:
                        fl = []
                        if nb >= 2:
                            fl += list(wo_units(nb - 2))
                        fl += list(stats_units(0, range(4 * nb, 4 * nb + 4)))
                        st3_emit(1, nb - 1, fillers=fl)
                        scale_emit(nb - 1)
                        st3_emit(
                            0, nb, fillers=stats_units(1, range(4 * nb, 4 * nb + 4))
                        )
                st3_emit(1, NB - 1, fillers=wo_units(NB - 2))
                scale_emit(NB - 1)
                for u in wo_units(NB - 1):
                    u()

    _split_waits(nc)
    return nc


_cache = {}


def _get_nc(causal: bool):
    if causal not in _cache:
        nc = bass.Bass(trn_type="TRN2")
        build(nc, causal=causal)
        _cache[causal] = nc
    return _cache[causal]


def _host_maskend():
    # mask_end columns for the stats tensor_mask_reduce: col j<4 = causal
    # end (exclusive) of the diag block's valid range when qb%4 == j;
    # col 4 = 512 (full block).
    p = np.arange(P)
    me = np.zeros((P, 5), dtype=np.float32)
    for j in range(4):
        me[:, j] = j * P + p + 1
    me[:, 4] = 512.0
    return me


def make_in_maps(np_inputs):
    bf_v = bool(int(os.environ.get("ATTN_BF16_V", "1")))
    Q = np.asarray(np_inputs["Q"], dtype=np.float32)
    K = np.asarray(np_inputs["K"], dtype=np.float32)
    V = np.asarray(np_inputs["V"], dtype=np.float32)
    W_Q = np.asarray(np_inputs["W_Q"], dtype=np.float32)
    W_K = np.asarray(np_inputs["W_K"], dtype=np.float32)
    W_V = np.asarray(np_inputs["W_V"], dtype=np.float32)
    W_O = np.asarray(np_inputs["W_O"], dtype=np.float32)

    qTh = np.ascontiguousarray(Q.T)
    kTh = np.ascontiguousarray(K.T)
    vTh = np.ascontiguousarray(V.T)
    if bf_v:
        vTh = vTh.astype(ml_dtypes.bfloat16)
    maskend = _host_maskend()
    p_ = np.arange(P)[:, None]
    j_ = np.arange(P)[None, :]
    maskf = np.where(p_ > j_, NEG, 0.0).astype(ml_dtypes.bfloat16)
    maskb = np.where(j_ > p_, NEG, 0.0).astype(ml_dtypes.bfloat16)
    ones_row = np.ones((1, S), dtype=np.float32)
    sel2 = np.zeros((33, P), dtype=np.float32)
    sel2[0, 0:DK] = 1.0
    sel2[32, DK:P] = 1.0

    scale = np.float32(1.0 / np.sqrt(DK))
    in_maps = []
    for c in range(NCORES):
        h0, h1 = 2 * c, 2 * c + 1
        wq2 = np.ascontiguousarray(
            np.concatenate([W_Q[h0] * scale, W_Q[h1] * scale], axis=1)
        ).astype(np.float32)
        wk2 = np.ascontiguousarray(np.concatenate([W_K[h0], W_K[h1]], axis=1))
        wv2 = np.ascontiguousarray(np.concatenate([W_V[h0], W_V[h1]], axis=1))
        if bf_v:
            wv2 = wv2.astype(ml_dtypes.bfloat16)
        wo2 = np.ascontiguousarray(W_O[P * c : P * (c + 1), :])
        in_maps.append(
            {
                "qT": qTh,
                "kT": kTh,
                "vT": vTh,
                "wq": wq2,
                "wk": wk2,
                "wv": wv2,
                "wo": wo2,
                "maskend": maskend,
                "maskf": maskf,
                "maskb": maskb,
                "onesr": ones_row,
                "sel2d": sel2,
                "zeros64": np.zeros((DK, S), dtype=np.float32),
            }
        )
    return in_maps


LAST_EXEC_NS = None


def kernel(Q, K, V, W_Q, W_K, W_V, W_O, mask):
    global LAST_EXEC_NS
    causal = bool(np.asarray(mask).item())
    nc = _get_nc(causal)
    in_maps = make_in_maps(
        dict(Q=Q, K=K, V=V, W_Q=W_Q, W_K=W_K, W_V=W_V, W_O=W_O)
    )

    trace = bool(int(os.environ.get("ATTN_TRACE", "0")))
    res = run_bass_kernel_spmd(
        nc, in_maps, core_ids=list(range(NCORES)), trace=trace
    )
    LAST_EXEC_NS = res.exec_time_ns

    out = np.zeros((S, D), dtype=np.float32)
    for c in range(NCORES):
        out += np.asarray(res.results[c]["y0"], dtype=np.float32)
    return out
